# revision 32
# baseline (speedup 1.0000x reference)
"""Cross-modal attention Trainium2 kernel (Bass/Tile), data-parallel over batch.

Per core (one batch element):
    q = img @ Wq.T + bq ; k = ts @ Wk.T + bk ; v = ts @ Wv.T + bv
    out = softmax(q @ k.T) @ v

Key algebraic fold: scores = q @ k.T decomposes as
    img (Wq.T Wk) ts.T  +  (bq.T Wk) ts.T  +  [terms constant over keys]
The key-constant terms cancel exactly under softmax, so with
    M = Wq.T @ Wk  [d1, d2]        (16 matmuls, NO weight transposes)
    c = Wk.T @ bq  [d2]            (16 tiny matmuls)
    qmT[d2, i] = sum_d1 M[d1,d2] imgT[d1,i] + c[d2]
    scores = qmT.T @ tsT
the whole k-projection (64 matmuls) and the Wq/Wk PE transposes are
eliminated, exactly (for any bq/bk).

Layout strategy (contraction dim must live on SBUF partitions):
  - img/ts are PE-transposed tile-wise into imgT/tsT [d, n] (fp32r).
    (fp32r/bf16 transpose modes are rejected or broken: mixed-dtype matmult
    is NCC-illegal, and fp32r stationary loads in transpose mode yield
    zeros on HW. So transposes run in plain fp32.)
  - M / c use ec-outer accumulation chains over per-ec-chunk weight DMAs
    (Wq/Wk interleaved on the ACT HWDGE queue) so the PE starts on them as
    soon as the first 512 KB of weights lands; Wv loads only after the img
    tiles so img gets full DMA bandwidth at startup.
  - scores S[i=128, j=2048] accumulated in PSUM as 2x [128,1024] chunk tiles
    (2 banks each, bufs=3) so the next q-tile's score matmuls can start as
    soon as the matching chunk's exp has drained.
  - row-softmax: per-512 DVE reduce_max (overlaps the score matmuls) + a
    4-wide combine, then 2x ACT Exp(bias=-max); the row-sum is reduced on
    DVE from the fp16 probs (keeps READ_ACCUMULATOR off the busy ACT queue
    and matches the PV numerator's quantization).
  - probs are written as fp16 and transposed via the DMA XBAR per-1024 chunk
    into probsT [j, i] for the PV matmul (off the PE); 1/sum is applied to
    the PV result on ACT (on DVE it queues behind the reduce_max chain and
    delays the PV psum recycle; finer XBAR chunks cost too much ACT
    sequencer time and starve the PE; the SP-queue XBAR returns wrong data).
  - fp32r matmuls throughout (12-bit significand, 1 cycle/row on PE); the PV
    matmul uses fp16 probs/v (11-bit significand).
"""

import numpy as np

import concourse.bass as bass
import concourse.mybir as mybir
import concourse.tile as tile
from concourse import bacc
from concourse.bass_utils import run_bass_kernel_spmd
from concourse.masks import make_identity

B, NQ, NK, D = 8, 2048, 2048, 512
P = 128
DC = D // P        # 4 contraction chunks
EC = D // P        # 4 output-dim chunks
TQ = NQ // P       # 16 query tiles
TK = NK // P       # 16 key tiles
JC = NK // 512     # 4 key chunks of 512 (scores free dim)
IC = NQ // 512     # 4 query chunks of 512 (projection free dim)

F32 = mybir.dt.float32
F32R = mybir.dt.float32r
F16 = mybir.dt.float16
BF16 = mybir.dt.bfloat16
AX = mybir.AxisListType.X
IDENT_FN = mybir.ActivationFunctionType.Identity
EXP_FN = mybir.ActivationFunctionType.Exp


def build():
    nc = bacc.Bacc(None, target_bir_lowering=False)

    img = nc.dram_tensor("img", [NQ, D], F32, kind="ExternalInput")
    ts = nc.dram_tensor("ts", [NK, D], F32, kind="ExternalInput")
    Wq = nc.dram_tensor("Wq", [D, D], F32R, kind="ExternalInput")
    Wk = nc.dram_tensor("Wk", [D, D], F32R, kind="ExternalInput")
    Wv = nc.dram_tensor("Wv", [D, D], F32, kind="ExternalInput")
    bq = nc.dram_tensor("bq", [D], F32R, kind="ExternalInput")
    bv = nc.dram_tensor("bv", [D], F32, kind="ExternalInput")
    out = nc.dram_tensor("out", [NQ, D], F32, kind="ExternalOutput")

    with tile.TileContext(nc) as tc:
        with (
            tc.tile_pool(name="const", bufs=1) as const_pool,
            tc.tile_pool(name="big", bufs=1) as big,
        ):
            ident = const_pool.tile([P, P], F32)
            make_identity(nc, ident)

            # biases: bq as [P, EC, 2] (e%128 on partitions, duplicated last
            # dim: fp32r matmuls need an even moving free count), bv
            # replicated [P, D]. gpsimd SWDGE queue keeps their tiny
            # descriptors off the bulk HWDGE queues.
            bq_sb = const_pool.tile([P, EC, 2], F32R)
            nc.gpsimd.dma_start(bq_sb[:, :, 0], bq.ap().rearrange("(c p) -> p c", p=P))
            nc.gpsimd.dma_start(bq_sb[:, :, 1], bq.ap().rearrange("(c p) -> p c", p=P))
            bv_sb = const_pool.tile([P, D], F32)
            nc.gpsimd.dma_start(bv_sb[:], bv.ap().partition_broadcast(P))
            c_sb = const_pool.tile([P, EC], F32)

            # persistent big operands
            qmT = big.tile([P, EC, NQ], F32R)  # qmT[p, ec, i] = (img@M)[i, ec*128+p]
            tsT = big.tile([P, DC, NK], F32R)  # tsT[p, dc, j] = ts[j, dc*128+p]
            v_sb = big.tile([P, TK, D], F16)   # v_sb[p, jt, e] = v[jt*128+p, e]
            M_sb = big.tile([P, DC, D], F32R)  # M_sb[p, d1c, d2] = M[d1c*128+p, d2]
            WvT = big.tile([P, DC, D], F32R)   # WvT[p, dc, e] = Wv[e, dc*128+p]

            def copy_alt(idx, dst, src):
                """psum->sbuf copy, alternating DVE / ACT to balance engines."""
                if idx % 2 == 0:
                    nc.vector.tensor_copy(dst, src)
                else:
                    nc.scalar.copy(dst, src)

            # ---- Phases 0-3: transposes, M/c fold, qm/v projections ----
            with (
                tc.tile_pool(name="wstage", bufs=1) as wstage,
                tc.tile_pool(name="xstage", bufs=8) as xstage,
                tc.tile_pool(name="xps", bufs=3, space="PSUM") as xps,
                tc.tile_pool(name="pps", bufs=4, space="PSUM") as pps,
                tc.tile_pool(name="cps", bufs=1, space="PSUM") as cps,
            ):
                WDRAM = {"q": Wq, "k": Wk, "v": Wv}
                w_sbs = {}

                def emit_w_dma(*wnames):
                    # ACT engine's HWDGE queue: weights flow in parallel with
                    # the img tiles on the SP queue. Chunked per-ec (and
                    # interleaved across the listed weights) so the ec-outer
                    # M/c matmul chains can start on chunk 0 while later
                    # chunks stream.
                    for wname in wnames:
                        wdt = F32 if wname == "v" else F32R
                        w_sbs[wname] = wstage.tile(
                            [P, EC, D], wdt, tag=f"wstage_{wname}",
                            name=f"w_sb_{wname}",
                        )
                    for ec in range(EC):
                        for wname in wnames:
                            nc.scalar.dma_start(
                                w_sbs[wname][:, ec, :],
                                WDRAM[wname]
                                .ap()
                                .rearrange("(ec p) d -> p ec d", p=P)[:, ec, :],
                            )

                def emit_c():
                    """c[d2] = sum_e Wk[e, d2] * bq[e]  -> c_sb [P, EC].

                    ec-outer so the chain starts as soon as Wk chunk 0 lands.
                    """
                    pc = cps.tile([P, EC, 2], F32, tag="cps")
                    for ec in range(EC):
                        for d2 in range(EC):
                            nc.tensor.matmul(
                                pc[:, d2, :],
                                w_sbs["k"][:, ec, d2 * P : (d2 + 1) * P],
                                bq_sb[:, ec, :],
                                start=(ec == 0),
                                stop=(ec == EC - 1),
                            )
                    nc.vector.tensor_copy(c_sb[:], pc[:, :, 0])

                def emit_m():
                    """M[d1, d2] = sum_e Wq[e, d1] Wk[e, d2] (both natural).

                    ec-outer across 4 concurrent psum accumulators so the
                    chain paces with the arriving Wq/Wk chunk DMAs.
                    """
                    pms = [
                        pps.tile([P, 512], F32, tag="pps", name=f"pm{d1}")
                        for d1 in range(DC)
                    ]
                    for ec in range(EC):
                        for d1 in range(DC):
                            nc.tensor.matmul(
                                pms[d1][:],
                                w_sbs["q"][:, ec, d1 * P : (d1 + 1) * P],
                                w_sbs["k"][:, ec, :],
                                start=(ec == 0),
                                stop=(ec == EC - 1),
                            )
                    for d1 in range(DC):
                        copy_alt(d1, M_sb[:, d1, :], pms[d1][:])

                def emit_wv_transpose():
                    w_sb = w_sbs["v"]
                    for ec in range(EC):
                        for dc in range(DC):
                            pw = xps.tile([P, P], F32, tag="xps")
                            nc.tensor.transpose(
                                pw[:], w_sb[:, ec, dc * P : (dc + 1) * P], ident[:]
                            )
                            copy_alt(
                                ec * DC + dc, WvT[:, dc, ec * P : (ec + 1) * P], pw[:]
                            )

                def emit_xt_chunk(src_dram, xT, ic):
                    """DMA 4 row-tiles of a 512-token chunk, PE-transpose into xT."""
                    for t in range(4):
                        it = 4 * ic + t
                        x_sb = xstage.tile([P, D], F32, tag="xstage")
                        nc.sync.dma_start(
                            x_sb[:], src_dram[it * P : (it + 1) * P, :]
                        )
                        for dc in range(DC):
                            px = xps.tile([P, P], F32, tag="xps")
                            nc.tensor.transpose(
                                px[:], x_sb[:, dc * P : (dc + 1) * P], ident[:]
                            )
                            copy_alt(dc, xT[:, dc, it * P : (it + 1) * P], px[:])

                def emit_qm_chunk(imgT, ic):
                    """qmT[:, :, ic*512:+512] = M.T @ imgT chunk + c (per-d2)."""
                    for d2 in range(EC):
                        pq = pps.tile([P, 512], F32, tag="pps")
                        for d1 in range(DC):
                            nc.tensor.matmul(
                                pq[:],
                                M_sb[:, d1, d2 * P : (d2 + 1) * P],
                                imgT[:, d1, ic * 512 : (ic + 1) * 512],
                                start=(d1 == 0),
                                stop=(d1 == DC - 1),
                            )
                        if d2 % 2 == 0:
                            nc.scalar.activation(
                                out=qmT[:, d2, ic * 512 : (ic + 1) * 512],
                                in_=pq[:],
                                func=IDENT_FN,
                                bias=c_sb[:, d2 : d2 + 1],
                                scale=1.0,
                            )
                        else:
                            nc.vector.tensor_scalar_add(
                                qmT[:, d2, ic * 512 : (ic + 1) * 512],
                                pq[:],
                                c_sb[:, d2 : d2 + 1],
                            )

                def emit_v_chunk(ic):
                    """v rows jt=4ic..4ic+3: v[j,e] = tsT_chunk.T @ WvT + bv."""
                    for t in range(4):
                        jt = 4 * ic + t
                        pv = pps.tile([P, 512], F32, tag="pps")
                        for dc in range(DC):
                            nc.tensor.matmul(
                                pv[:],
                                tsT[:, dc, jt * P : (jt + 1) * P],
                                WvT[:, dc, :],
                                start=(dc == 0),
                                stop=(dc == DC - 1),
                            )
                        nc.vector.tensor_add(v_sb[:, jt, :], pv[:], bv_sb[:])

                with tc.tile_pool(name="xt", bufs=1) as xtp:
                    imgT = xtp.tile([P, DC, NQ], F32R, tag="xT")
                    # DMA issue order sets arrival order: Wq/Wk first on the
                    # ACT queue (M/c unblock early), img on the SP queue.
                    emit_w_dma("q", "k")
                    emit_xt_chunk(img, imgT, 0)
                    emit_c()
                    emit_xt_chunk(img, imgT, 1)
                    emit_m()
                    emit_xt_chunk(img, imgT, 2)
                    emit_qm_chunk(imgT, 0)
                    # Wv loads only now: it is not needed until emit_wv_transpose
                    # consumes it (~mid phase B), and keeping it off the queues
                    # early gives the img tiles full DMA bandwidth at startup.
                    emit_w_dma("v")
                    emit_xt_chunk(img, imgT, 3)
                    emit_wv_transpose()
                    emit_qm_chunk(imgT, 1)
                    emit_qm_chunk(imgT, 2)
                    emit_qm_chunk(imgT, 3)

                for ic in range(IC):
                    emit_xt_chunk(ts, tsT, ic)
                    if ic >= 1:
                        emit_v_chunk(ic - 1)
                emit_v_chunk(IC - 1)

            # ---- Phase 4: attention (software-pipelined by one q-tile) ----
            with (
                tc.tile_pool(name="sps", bufs=3, space="PSUM") as sps,
                tc.tile_pool(name="ops", bufs=2, space="PSUM") as ops,
                tc.tile_pool(name="soft", bufs=3) as soft,
                tc.tile_pool(name="outp", bufs=3) as outp,
            ):
                stash = {}

                def emit_scores_softmax(qt):
                    pmax = soft.tile([P, JC], F32, tag="pmax")
                    chunks = []
                    for jc2 in range(2):
                        Sc = sps.tile([P, 1024], F32, tag="S")
                        chunks.append(Sc)
                        for h in range(2):
                            jc = 2 * jc2 + h
                            for ec in range(EC):
                                nc.tensor.matmul(
                                    Sc[:, h * 512 : (h + 1) * 512],
                                    qmT[:, ec, qt * P : (qt + 1) * P],
                                    tsT[:, ec, jc * 512 : (jc + 1) * 512],
                                    start=(ec == 0),
                                    stop=(ec == EC - 1),
                                )
                            # chunk max overlaps the next chunk's matmuls
                            nc.vector.reduce_max(
                                pmax[:, jc : jc + 1],
                                Sc[:, h * 512 : (h + 1) * 512],
                                axis=AX,
                            )
                    negmax = soft.tile([P, 1], F32, tag="negmax")
                    nc.vector.reduce_max(negmax[:], pmax[:], axis=AX, negate=True)
                    # bufs=4 (not the pool's 3): the steady-state DVE
                    # row-sum frees probs(qt-3) late enough that exp(qt)
                    # occasionally stalls on the recycle; a 4th buffer adds a
                    # full round of slack (SBUF has ample headroom).
                    probs = soft.tile([P, NK], F16, tag="probs", bufs=4)
                    # bufs=4: the XBAR transpose of tile N otherwise waits
                    # on tile N-3's PV to release its probsT buffer, an edge
                    # on the serialized XBAR chain.
                    probsT = soft.tile([P, TK, P], F16, tag="probsT", bufs=4)
                    # Tail tiles keep the ACT accum_out row-sum: their pv is on
                    # the critical path and the 1.5-1.8 us DVE reduce over
                    # probs would land inside it. Steady tiles compute the
                    # row-sum on DVE in emit_pv (fully overlapped), keeping
                    # READ_ACCUMULATOR off the busy ACT queue.
                    tail = qt >= TQ - 2
                    rowsum4 = (
                        soft.tile([P, 2], F32, tag="rowsum4", name="rowsum4")
                        if tail
                        else None
                    )
                    # exp + XBAR transpose per 1024: coarse chunks keep the ACT
                    # sequencer overhead down (finer splits starve the PE; the
                    # SP-queue XBAR transpose returns wrong data, so everything
                    # stays on the ACT HWDGE queue). No accum_out: the row-sum
                    # is computed on DVE from the fp16 probs in emit_pv, which
                    # drops the ACT READ_ACCUMULATOR instructions and matches
                    # the PV numerator's fp16 quantization exactly.
                    for jc2 in range(2):
                        nc.scalar.activation(
                            out=probs[:, jc2 * 1024 : (jc2 + 1) * 1024],
                            in_=chunks[jc2][:],
                            func=EXP_FN,
                            bias=negmax[:],
                            scale=1.0,
                            accum_out=(
                                rowsum4[:, jc2 : jc2 + 1] if tail else None
                            ),
                        )
                        nc.scalar.dma_start_transpose(
                            probsT[:, jc2 * 8 : (jc2 + 1) * 8, :],
                            probs[:, jc2 * 1024 : (jc2 + 1) * 1024],
                        )
                    stash[qt] = (probsT, probs, rowsum4)

                def emit_pv(qt, tail=False):
                    probsT, probs, rowsum4 = stash.pop(qt)
                    rowsum = soft.tile([P, 1], F32, tag="rowsum")
                    if rowsum4 is not None:
                        nc.vector.reduce_sum(rowsum[:], rowsum4[:], axis=AX)
                    else:
                        nc.vector.reduce_sum(rowsum[:], probs[:], axis=AX)
                    recip = soft.tile([P, 1], F32, tag="recip")
                    nc.vector.reciprocal(recip[:], rowsum[:])
                    po = ops.tile([P, D], F32, tag="po")
                    for jt in range(TK):
                        nc.tensor.matmul(
                            po[:],
                            probsT[:, jt, :],
                            v_sb[:, jt, :],
                            start=(jt == 0),
                            stop=(jt == TK - 1),
                        )
                    o_sb = outp.tile([P, D], F32, tag="o")
                    # Steady state: ACT (on DVE this queues behind the
                    # reduce_max chain and delays the PV psum recycle; gpsimd
                    # cannot read PSUM at all). Tail tiles: DVE — it is idle
                    # there, and an ACT mul would sit between the last tiles'
                    # XBAR transposes on the ACT FIFO, stalling the final PV.
                    if tail:
                        # halves: the first half's store overlaps the second
                        # half's multiply, trimming the final drain.
                        for h in range(2):
                            nc.vector.tensor_scalar_mul(
                                o_sb[:, h * 256 : (h + 1) * 256],
                                po[:, h * 256 : (h + 1) * 256],
                                recip[:],
                            )
                            nc.sync.dma_start(
                                out[qt * P : (qt + 1) * P, h * 256 : (h + 1) * 256],
                                o_sb[:, h * 256 : (h + 1) * 256],
                            )
                    else:
                        nc.scalar.mul(out=o_sb[:], in_=po[:], mul=recip[:])
                        nc.sync.dma_start(out[qt * P : (qt + 1) * P, :], o_sb[:])

                for qt in range(TQ):
                    emit_scores_softmax(qt)
                    if qt >= 2:
                        emit_pv(qt - 2)
                emit_pv(TQ - 2, tail=True)
                emit_pv(TQ - 1, tail=True)

    nc.compile()
    return nc


_NC_CACHE = None


def _get_nc():
    global _NC_CACHE
    if _NC_CACHE is None:
        _NC_CACHE = build()
    return _NC_CACHE


def run(inputs: dict, trace: bool = False):
    """Run on 8 cores, batch-parallel. Returns (out [B,NQ,D], BassKernelResults)."""
    nc = _get_nc()
    in_maps = []
    for b in range(B):
        in_maps.append(
            {
                "img": np.ascontiguousarray(np.asarray(inputs["img_feats"][b], np.float32)),
                "ts": np.ascontiguousarray(np.asarray(inputs["ts_feats"][b], np.float32)),
                "Wq": np.asarray(inputs["Wq"], np.float32),
                "Wk": np.asarray(inputs["Wk"], np.float32),
                "Wv": np.asarray(inputs["Wv"], np.float32),
                "bq": np.asarray(inputs["bq"], np.float32),
                "bv": np.asarray(inputs["bv"], np.float32),
            }
        )
    res = run_bass_kernel_spmd(nc, in_maps, core_ids=list(range(B)), trace=trace)
    full = np.stack([res.results[b]["out"] for b in range(B)], axis=0)
    return full, res


def kernel(**inputs) -> np.ndarray:
    full, _ = run(inputs, trace=False)
    return full
